# revision 17
# baseline (speedup 1.0000x reference)
"""Bass/Trainium2 kernel for GQA decode attention (fused K-projection form).

Reference computation (per problem spec):
  x = x_pre[:, -1, :]                               # [16, 4096]
  xq = (x @ wq.T) -> [b, 32, 128]
  qt[b,h,:] = xq[b,h,:] @ wk[kv(h)*128:+128, :]     # [b, 32, 4096]
  scores = qt . x_pre / sqrt(128)                   # [b, 32, 2048]
  attn = softmax_t(scores)
  ctx[b,h,:] = sum_t attn[b,h,t] * x_pre[b,t,:]     # [b, 32, 4096]  (lazy-V)
  out[b,h,d] = sum_D ctx[b,h,D] * wv[kv(h)*128+d,D] # [b, 32, 128]
  y = out.flat @ wo.T                               # [16, 4096]

Sharding (8 cores): batch-parallel attention (2 batches/core) +
head-parallel projections (4 heads = 1 kv head/core), exchanged with two
AllToAll collectives. wo is column-sharded (contraction dim); host sums
the 8 partial y outputs.
"""

import math

import numpy as np

import concourse.bass as bass
import concourse.mybir as mybir
import concourse.tile as tile
from concourse import bacc
from concourse.bass_utils import run_bass_kernel_spmd
from concourse.masks import make_identity
from concourse.tile import add_dep_helper

F32 = mybir.dt.float32
NC = 8
BSZ = 16
SEQ = 2048
DIM = 4096
NH = 32
HD = 128
B_LOC = 2        # batches per core
HL = 4           # local heads per core
NT = SEQ // 128  # 16 t-tiles per batch
NDC = DIM // 128 # 32 D-chunks
SCALE = 1.0 / math.sqrt(HD)


def build_program(trace_label="", debug=False, nocc=False, skel=False):
    nc = bacc.Bacc("TRN2", target_bir_lowering=False, debug=False)

    xp = nc.dram_tensor("xp", [B_LOC, SEQ, DIM], F32, kind="ExternalInput")
    xl = nc.dram_tensor("xl", [BSZ, DIM], F32, kind="ExternalInput")
    wq = nc.dram_tensor("wq", [HL * HD, DIM], F32, kind="ExternalInput")
    wk = nc.dram_tensor("wk", [HD, DIM], F32, kind="ExternalInput")
    wv = nc.dram_tensor("wv", [HD, DIM], F32, kind="ExternalInput")
    wo = nc.dram_tensor("wo", [DIM, HL * HD], F32, kind="ExternalInput")
    y = nc.dram_tensor("y", [BSZ, DIM], F32, kind="ExternalOutput")
    if debug:
        dbg_q = nc.dram_tensor("dbg_q", [B_LOC * NH, DIM], F32,
                               kind="ExternalOutput")
        dbg_ctx = nc.dram_tensor("dbg_ctx", [BSZ * HL, DIM], F32,
                                 kind="ExternalOutput")
        dbg_sc = nc.dram_tensor("dbg_sc", [B_LOC * NH, 128], F32,
                                kind="ExternalOutput")
        dbg_xq = nc.dram_tensor("dbg_xq", [BSZ, HL * HD], F32,
                                kind="ExternalOutput")
        dbg_qs = nc.dram_tensor("dbg_qs", [BSZ, DIM], F32,
                                kind="ExternalOutput")

    rg = [list(range(NC))]

    with tile.TileContext(nc) as tc:
        with (
            tc.tile_pool(name="persist", bufs=1) as pers,
            tc.tile_pool(name="dram", bufs=1, space="DRAM") as dram,
        ):
            ident = pers.tile([128, 128], F32)
            make_identity(nc, ident)

            # DRAM exchange buffers
            a2a1_in = dram.tile([NC * B_LOC * HL, DIM], F32)   # [64, 4096]
            a2a1_out = dram.tile([NC * B_LOC * HL, DIM], F32)
            a2a2_in = dram.tile([NC * B_LOC * HL, DIM], F32)
            a2a2_out = dram.tile([NC * B_LOC * HL, DIM], F32)

            stage_dmas1 = []
            stage_dmas2 = []
            # ---------------- Phase 1: q-tilde for local heads, all batches
            with (
                tc.tile_pool(name="p1", bufs=2) as p1,
                tc.tile_pool(name="p1w", bufs=1) as p1w,
                tc.tile_pool(name="p1ps", bufs=2, space="PSUM") as p1ps,
            ):
                xl_sb = p1w.tile([BSZ, DIM], F32)
                nc.sync.dma_start(out=xl_sb, in_=xl[:, :])
                wk_sb = p1w.tile([HD, DIM], F32)
                nc.sync.dma_start(out=wk_sb, in_=wk[:, :])

                # xT: [128 D x 16 b] per D-chunk
                xT = p1w.tile([128, NDC * BSZ], F32)
                for c in range(NDC):
                    tp = p1ps.tile([128, BSZ], F32, tag="tp1")
                    nc.tensor.transpose(tp, xl_sb[:, c * 128:(c + 1) * 128],
                                        ident[0:BSZ, 0:BSZ])
                    nc.vector.tensor_copy(out=xT[:, c * BSZ:(c + 1) * BSZ], in_=tp)

                # wqT: per D-chunk c: [128 D x 512 hd]
                wqT = p1w.tile([128, NDC * HL * HD], F32)
                for m in range(HL):
                    wq_sb = p1.tile([128, DIM], F32, tag="wqnat")
                    nc.sync.dma_start(out=wq_sb, in_=wq[m * 128:(m + 1) * 128, :])
                    for c in range(NDC):
                        tp = p1ps.tile([128, 128], F32, tag="tp1")
                        nc.tensor.transpose(tp, wq_sb[:, c * 128:(c + 1) * 128],
                                            ident)
                        nc.vector.tensor_copy(
                            out=wqT[:, c * 512 + m * 128: c * 512 + (m + 1) * 128],
                            in_=tp)

                # xq = x @ wq_slice.T : accumulate over D-chunks -> [16 b, 512 hd]
                xq_ps = p1ps.tile([BSZ, HL * HD], F32, tag="xq")
                for c in range(NDC):
                    nc.tensor.matmul(xq_ps, xT[:, c * BSZ:(c + 1) * BSZ],
                                     wqT[:, c * 512:(c + 1) * 512],
                                     start=(c == 0), stop=(c == NDC - 1))
                xq_sb = p1w.tile([BSZ, HL * HD], F32)
                nc.vector.tensor_copy(out=xq_sb, in_=xq_ps)
                if debug:
                    nc.sync.dma_start(out=dbg_xq[:, :], in_=xq_sb)

                # xqT: [128 d x 16 b] per local head
                xqT = p1w.tile([128, HL * BSZ], F32)
                for m in range(HL):
                    tp = p1ps.tile([128, BSZ], F32, tag="tp1")
                    nc.tensor.transpose(tp, xq_sb[:, m * 128:(m + 1) * 128],
                                        ident[0:BSZ, 0:BSZ])
                    nc.vector.tensor_copy(out=xqT[:, m * BSZ:(m + 1) * BSZ], in_=tp)

                # qt[h] = xq[:,h,:] @ wk_kv  (scaled) -> staged [64, 4096]
                # row layout = h_loc*16 + b
                for m in range(HL):
                    qstage = p1.tile([BSZ, DIM], F32, tag="qstage")
                    for j in range(8):
                        q_ps = p1ps.tile([BSZ, 512], F32, tag="qps")
                        nc.tensor.matmul(q_ps, xqT[:, m * BSZ:(m + 1) * BSZ],
                                         wk_sb[:, j * 512:(j + 1) * 512],
                                         start=True, stop=True)
                        nc.scalar.mul(
                            out=qstage[:, j * 512:(j + 1) * 512],
                            in_=q_ps, mul=SCALE)
                    d = nc.sync.dma_start(
                        out=a2a1_in.rearrange("(r b h) d -> h r b d",
                                              r=NC, b=B_LOC)[m],
                        in_=qstage)
                    stage_dmas1.append(d)
                    if debug and m == 0:
                        nc.sync.dma_start(out=dbg_qs[:, :], in_=qstage)


            if not nocc:
                cc1 = nc.gpsimd.collective_compute(
                    "AllToAll", mybir.AluOpType.bypass,
                    ins=[a2a1_in.opt()], outs=[a2a1_out.opt()], replica_groups=rg)
                for d in stage_dmas1:
                    add_dep_helper(cc1.ins, d.ins, reason="a2a1 input ready")

            # qT per local batch: [128 D x 32 h] per D-chunk
            # a2a1_out row = src_r*8 + b_loc*4 + h_loc ; head = 4*src_r + h_loc
            qT = [pers.tile([128, NDC * NH], F32, tag=f"qT{b}", name=f"qT{b}")
                  for b in range(B_LOC)]
            with (
                tc.tile_pool(name="qnat", bufs=2) as qnatp,
                tc.tile_pool(name="qnps", bufs=2, space="PSUM") as qnps,
            ):
                for b in range(B_LOC):
                    qnat = qnatp.tile([NH, DIM], F32, tag="qnat")
                    d = nc.sync.dma_start(
                        out=qnat,
                        in_=a2a1_out.rearrange("(r b h) d -> b r h d",
                                               r=NC, b=B_LOC)[b])
                    if not nocc:
                        add_dep_helper(d.ins, cc1.ins, reason="a2a1 done")
                    if debug:
                        nc.sync.dma_start(out=dbg_q[b * NH:(b + 1) * NH, :],
                                          in_=qnat)
                    for c in range(NDC):
                        tp = qnps.tile([128, NH], F32, tag="tpq")
                        nc.tensor.transpose(tp, qnat[:, c * 128:(c + 1) * 128],
                                            ident[0:NH, 0:NH])
                        nc.vector.tensor_copy(
                            out=qT[b][:, c * NH:(c + 1) * NH], in_=tp)

            # ---------------- Phase 2: streaming attention per local batch
            with (
                tc.tile_pool(name="xpool", bufs=6) as xpool,
                tc.tile_pool(name="xtpool", bufs=1) as xtpool,
                tc.tile_pool(name="attn", bufs=3) as apool,
                tc.tile_pool(name="small", bufs=2) as smallp,
                tc.tile_pool(name="ctx_sb", bufs=1) as ctxsbp,
                tc.tile_pool(name="tps", bufs=2, space="PSUM") as tps,
                tc.tile_pool(name="scps", bufs=2, space="PSUM") as scps,
                tc.tile_pool(name="ctxps", bufs=2, space="PSUM") as ctxps,
            ):
                for b in range(B_LOC):
                    ctx_ps = ctxps.tile([128, 1024], F32, tag="ctx")
                    sums = smallp.tile([NH, 4], F32, tag="sums")
                    for ch in range(4):
                        xts = []
                        for tt in range(4):
                            t = ch * 4 + tt
                            x_sb = xpool.tile([128, DIM], F32, tag="x",
                                              name=f"x{b}_{t}")
                            nc.sync.dma_start(
                                out=x_sb, in_=xp[b, t * 128:(t + 1) * 128, :])
                            xts.append(x_sb)
                        xtt = xtpool.tile([128, NDC * 512], F32, tag="xt")
                        xv = xtt.rearrange("p (c t) -> p c t", c=NDC)
                        for tt in range(4):
                            for cg in range(NDC // 4):
                                tp = tps.tile([128, 512], F32, tag="tp2")
                                for cc in range(4):
                                    c = cg * 4 + cc
                                    nc.tensor.transpose(
                                        tp[:, cc * 128:(cc + 1) * 128],
                                        xts[tt][:, c * 128:(c + 1) * 128], ident)
                                nc.vector.tensor_copy(
                                    out=xv[:, cg * 4:(cg + 1) * 4,
                                           tt * 128:(tt + 1) * 128],
                                    in_=tp.rearrange("p (a q) -> p a q", a=4))
                        if skel:
                            continue
                        sc_ps = scps.tile([NH, 512], F32, tag="sc")
                        for c in range(NDC):
                            nc.tensor.matmul(sc_ps,
                                             qT[b][:, c * NH:(c + 1) * NH],
                                             xtt[:, c * 512:(c + 1) * 512],
                                             start=(c == 0), stop=(c == NDC - 1))
                        attn_sb = apool.tile([NH, 512], F32, tag="attn")
                        nc.scalar.activation(out=attn_sb, in_=sc_ps,
                                             func=mybir.ActivationFunctionType.Exp,
                                             accum_out=sums[:, ch:ch + 1])
                        at_ps = tps.tile([128, 512], F32, tag="tp2")
                        for tt in range(4):
                            nc.tensor.transpose(
                                at_ps[:, tt * NH:(tt + 1) * NH],
                                attn_sb[:, tt * 128:(tt + 1) * 128],
                                ident[0:NH, 0:NH])
                        at_sb = apool.tile([128, 4 * NH], F32, tag="attnT")
                        nc.vector.tensor_copy(out=at_sb, in_=at_ps[:, 0:4 * NH])
                        for tt in range(4):
                            for g in range(4):
                                for jj in (g, g + 4):
                                    nc.tensor.matmul(
                                        ctx_ps[g * 32:(g + 1) * 32,
                                               (jj // 4) * 512:(jj // 4 + 1) * 512],
                                        at_sb[:, tt * NH:(tt + 1) * NH],
                                        xts[tt][:, jj * 512:(jj + 1) * 512],
                                        start=(ch == 0 and tt == 0),
                                        stop=(ch == 3 and tt == 3),
                                        tile_position=(0, g * 32))
                    # finalize batch: 1/rowsum, scale, stage for exchange
                    ssum = smallp.tile([NH, 1], F32, tag="ssum")
                    nc.vector.reduce_sum(out=ssum, in_=sums,
                                         axis=mybir.AxisListType.X)
                    rsum = smallp.tile([128, 1], F32, tag="rsum")
                    nc.vector.reciprocal(out=rsum[0:NH], in_=ssum)
                    for g in range(1, 4):
                        nc.vector.tensor_copy(out=rsum[g * 32:(g + 1) * 32],
                                              in_=rsum[0:NH])
                    ctx_sb = ctxsbp.tile([NH, DIM], F32, tag="ctxsb")
                    for j in range(8):
                        g = j % 4
                        nc.vector.tensor_scalar_mul(
                            ctx_sb[:, j * 512:(j + 1) * 512],
                            ctx_ps[g * 32:(g + 1) * 32,
                                   (j // 4) * 512:(j // 4 + 1) * 512],
                            rsum[g * 32:(g + 1) * 32, 0:1])
                    # ship: dest rank r gets heads 4r..4r+3 ; row r*8 + b*4 + h
                    d = nc.sync.dma_start(
                        out=a2a2_in.rearrange("(r b h) d -> r b h d",
                                              r=NC, b=B_LOC)[:, b],
                        in_=ctx_sb)
                    stage_dmas2.append(d)

            if not nocc:
                cc2 = nc.gpsimd.collective_compute(
                    "AllToAll", mybir.AluOpType.bypass,
                    ins=[a2a2_in.opt()], outs=[a2a2_out.opt()], replica_groups=rg)
                for d in stage_dmas2:
                    add_dep_helper(cc2.ins, d.ins, reason="a2a2 input ready")

            # ---------------- Phase 3: output projection (local heads, all b)
            with (
                tc.tile_pool(name="p3", bufs=2) as p3,
                tc.tile_pool(name="p3w", bufs=1) as p3w,
                tc.tile_pool(name="p3ps", bufs=2, space="PSUM") as p3ps,
            ):
                # a2a2_out row = src_r*8 + b_loc*4 + h = b*4 + h  (b=2*src_r+b_loc)
                ctxg = p3w.tile([BSZ * HL, DIM], F32)
                d = nc.sync.dma_start(out=ctxg, in_=a2a2_out[:, :])
                if not nocc:
                    add_dep_helper(d.ins, cc2.ins, reason="a2a2 done")
                if debug:
                    nc.sync.dma_start(out=dbg_ctx[:, :], in_=ctxg)
                ctxgT = p3w.tile([128, NDC * BSZ * HL], F32)
                for c in range(NDC):
                    tp = p3ps.tile([128, 128], F32, tag="tp3")
                    nc.tensor.transpose(tp[:, 0:BSZ * HL],
                                        ctxg[:, c * 128:(c + 1) * 128],
                                        ident[0:BSZ * HL, 0:BSZ * HL])
                    nc.vector.tensor_copy(out=ctxgT[:, c * 64:(c + 1) * 64],
                                          in_=tp[:, 0:64])

                wv_sb = p3w.tile([HD, DIM], F32)
                nc.sync.dma_start(out=wv_sb, in_=wv[:, :])
                wvT = p3w.tile([128, DIM], F32)
                for c in range(NDC):
                    tp = p3ps.tile([128, 128], F32, tag="tp3")
                    nc.tensor.transpose(tp, wv_sb[:, c * 128:(c + 1) * 128], ident)
                    nc.vector.tensor_copy(out=wvT[:, c * 128:(c + 1) * 128], in_=tp)

                # out[b*4+h, d] accumulation over D-chunks
                op_ps = p3ps.tile([BSZ * HL, HD], F32, tag="op")
                for c in range(NDC):
                    nc.tensor.matmul(op_ps, ctxgT[:, c * 64:(c + 1) * 64],
                                     wvT[:, c * 128:(c + 1) * 128],
                                     start=(c == 0), stop=(c == NDC - 1))
                op_sb = p3w.tile([BSZ * HL, HD], F32)
                nc.vector.tensor_copy(out=op_sb, in_=op_ps)
                # outT [128 d x 64 (b*4+h)]
                otp = p3ps.tile([128, 128], F32, tag="tp3")
                nc.tensor.transpose(otp[:, 0:64], op_sb,
                                    ident[0:BSZ * HL, 0:BSZ * HL])
                outT = p3w.tile([128, BSZ * HL], F32)
                nc.vector.tensor_copy(out=outT, in_=otp[:, 0:64])

                # woT: per h_rel m: [128 d x 4096 j]
                woT = p3w.tile([128, HL * DIM], F32)
                for jt in range(NDC):
                    wo_sb = p3.tile([128, HL * HD], F32, tag="wonat")
                    nc.sync.dma_start(out=wo_sb, in_=wo[jt * 128:(jt + 1) * 128, :])
                    for m in range(HL):
                        tp = p3ps.tile([128, 128], F32, tag="tp3")
                        nc.tensor.transpose(tp, wo_sb[:, m * 128:(m + 1) * 128],
                                            ident)
                        nc.vector.tensor_copy(
                            out=woT[:, m * DIM + jt * 128: m * DIM + (jt + 1) * 128],
                            in_=tp)

                # y partial [16 b, 4096 j]
                y_sb = p3w.tile([BSZ, DIM], F32)
                oT = outT.rearrange("p (b h) -> p h b", h=HL)
                for jc in range(8):
                    y_ps = p3ps.tile([BSZ, 512], F32, tag="yps")
                    for m in range(HL):
                        nc.tensor.matmul(y_ps, oT[:, m, :],
                                         woT[:, m * DIM + jc * 512:
                                             m * DIM + (jc + 1) * 512],
                                         start=(m == 0), stop=(m == HL - 1))
                    nc.vector.tensor_copy(out=y_sb[:, jc * 512:(jc + 1) * 512],
                                          in_=y_ps)
                nc.sync.dma_start(out=y[:, :], in_=y_sb)

    nc.finalize()
    return nc


_PROGRAM_CACHE = {}


def kernel(x_pre, wq, wk, wv, wo, _trace=False, _tmpdir=None):
    x_pre = np.ascontiguousarray(np.asarray(x_pre, dtype=np.float32))
    wq = np.asarray(wq, dtype=np.float32)
    wk = np.asarray(wk, dtype=np.float32)
    wv = np.asarray(wv, dtype=np.float32)
    wo = np.asarray(wo, dtype=np.float32)

    if "nc" not in _PROGRAM_CACHE:
        _PROGRAM_CACHE["nc"] = build_program()
    nc = _PROGRAM_CACHE["nc"]

    xl = np.ascontiguousarray(x_pre[:, -1, :])
    in_maps = []
    for i in range(NC):
        in_maps.append({
            "xp": np.ascontiguousarray(x_pre[2 * i:2 * i + 2]),
            "xl": xl,
            "wq": np.ascontiguousarray(wq[512 * i:512 * (i + 1), :]),
            "wk": np.ascontiguousarray(wk[128 * i:128 * (i + 1), :]),
            "wv": np.ascontiguousarray(wv[128 * i:128 * (i + 1), :]),
            "wo": np.ascontiguousarray(wo[:, 512 * i:512 * (i + 1)]),
        })

    kwargs = {}
    if _trace:
        kwargs = dict(trace=True, trace_cores=[0])
    if _tmpdir is not None:
        kwargs["tmpdir"] = _tmpdir
    res = run_bass_kernel_spmd(nc, in_maps, core_ids=list(range(NC)), **kwargs)
    y = np.zeros((BSZ, DIM), np.float32)
    for i in range(NC):
        y += res.results[i]["y"]
    if _trace:
        print("HW exec time:", res.exec_time_ns, "ns")
    return y.reshape(BSZ, 1, DIM)


# revision 18
# speedup vs baseline: 1.0092x; 1.0092x over previous
"""Bass/Trainium2 kernel for GQA decode attention (fused K-projection form).

Reference computation (per problem spec):
  x = x_pre[:, -1, :]                               # [16, 4096]
  xq = (x @ wq.T) -> [b, 32, 128]
  qt[b,h,:] = xq[b,h,:] @ wk[kv(h)*128:+128, :]     # [b, 32, 4096]
  scores = qt . x_pre / sqrt(128)                   # [b, 32, 2048]
  attn = softmax_t(scores)
  ctx[b,h,:] = sum_t attn[b,h,t] * x_pre[b,t,:]     # [b, 32, 4096]  (lazy-V)
  out[b,h,d] = sum_D ctx[b,h,D] * wv[kv(h)*128+d,D] # [b, 32, 128]
  y = out.flat @ wo.T                               # [16, 4096]

Sharding (8 cores): batch-parallel attention (2 batches/core) +
head-parallel projections (4 heads = 1 kv head/core), exchanged with two
AllToAll collectives. wo is column-sharded (contraction dim); host sums
the 8 partial y outputs.
"""

import math

import numpy as np

import concourse.bass as bass
import concourse.mybir as mybir
import concourse.tile as tile
from concourse import bacc
from concourse.bass_utils import run_bass_kernel_spmd
from concourse.masks import make_identity
from concourse.tile import add_dep_helper

F32 = mybir.dt.float32
NC = 8
BSZ = 16
SEQ = 2048
DIM = 4096
NH = 32
HD = 128
B_LOC = 2        # batches per core
HL = 4           # local heads per core
NT = SEQ // 128  # 16 t-tiles per batch
NDC = DIM // 128 # 32 D-chunks
SCALE = 1.0 / math.sqrt(HD)


def build_program(trace_label="", debug=False, nocc=False, skel=False):
    nc = bacc.Bacc("TRN2", target_bir_lowering=False, debug=False)

    xp = nc.dram_tensor("xp", [B_LOC, SEQ, DIM], F32, kind="ExternalInput")
    xl = nc.dram_tensor("xl", [BSZ, DIM], F32, kind="ExternalInput")
    wq = nc.dram_tensor("wq", [HL * HD, DIM], F32, kind="ExternalInput")
    wk = nc.dram_tensor("wk", [HD, DIM], F32, kind="ExternalInput")
    wv = nc.dram_tensor("wv", [HD, DIM], F32, kind="ExternalInput")
    wo = nc.dram_tensor("wo", [DIM, HL * HD], F32, kind="ExternalInput")
    y = nc.dram_tensor("y", [BSZ, DIM], F32, kind="ExternalOutput")
    if debug:
        dbg_q = nc.dram_tensor("dbg_q", [B_LOC * NH, DIM], F32,
                               kind="ExternalOutput")
        dbg_ctx = nc.dram_tensor("dbg_ctx", [BSZ * HL, DIM], F32,
                                 kind="ExternalOutput")
        dbg_sc = nc.dram_tensor("dbg_sc", [B_LOC * NH, 128], F32,
                                kind="ExternalOutput")
        dbg_xq = nc.dram_tensor("dbg_xq", [BSZ, HL * HD], F32,
                                kind="ExternalOutput")
        dbg_qs = nc.dram_tensor("dbg_qs", [BSZ, DIM], F32,
                                kind="ExternalOutput")

    rg = [list(range(NC))]

    with tile.TileContext(nc) as tc:
        with (
            tc.tile_pool(name="persist", bufs=1) as pers,
            tc.tile_pool(name="dram", bufs=1, space="DRAM") as dram,
        ):
            ident = pers.tile([128, 128], F32)
            make_identity(nc, ident)

            # DRAM exchange buffers
            a2a1_in = dram.tile([NC * B_LOC * HL, DIM], F32)   # [64, 4096]
            a2a1_out = dram.tile([NC * B_LOC * HL, DIM], F32)
            a2a2_in = dram.tile([NC * B_LOC * HL, DIM], F32)
            a2a2_out = dram.tile([NC * B_LOC * HL, DIM], F32)

            stage_dmas1 = []
            stage_dmas2 = []
            # ---------------- Phase 1: q-tilde for local heads, all batches
            with (
                tc.tile_pool(name="p1", bufs=2) as p1,
                tc.tile_pool(name="p1w", bufs=1) as p1w,
                tc.tile_pool(name="p1ps", bufs=2, space="PSUM") as p1ps,
            ):
                xl_sb = p1w.tile([BSZ, DIM], F32)
                nc.sync.dma_start(out=xl_sb, in_=xl[:, :])
                wk_sb = p1w.tile([HD, DIM], F32)
                nc.sync.dma_start(out=wk_sb, in_=wk[:, :])

                # xT: [128 D x 16 b] per D-chunk
                xT = p1w.tile([128, NDC * BSZ], F32)
                for c in range(NDC):
                    tp = p1ps.tile([128, BSZ], F32, tag="tp1")
                    nc.tensor.transpose(tp, xl_sb[:, c * 128:(c + 1) * 128],
                                        ident[0:BSZ, 0:BSZ])
                    nc.vector.tensor_copy(out=xT[:, c * BSZ:(c + 1) * BSZ], in_=tp)

                # wqT: per D-chunk c: [128 D x 512 hd]
                wqT = p1w.tile([128, NDC * HL * HD], F32)
                for m in range(HL):
                    wq_sb = p1.tile([128, DIM], F32, tag="wqnat")
                    nc.sync.dma_start(out=wq_sb, in_=wq[m * 128:(m + 1) * 128, :])
                    for c in range(NDC):
                        tp = p1ps.tile([128, 128], F32, tag="tp1")
                        nc.tensor.transpose(tp, wq_sb[:, c * 128:(c + 1) * 128],
                                            ident)
                        nc.vector.tensor_copy(
                            out=wqT[:, c * 512 + m * 128: c * 512 + (m + 1) * 128],
                            in_=tp)

                # xq = x @ wq_slice.T : accumulate over D-chunks -> [16 b, 512 hd]
                xq_ps = p1ps.tile([BSZ, HL * HD], F32, tag="xq")
                for c in range(NDC):
                    nc.tensor.matmul(xq_ps, xT[:, c * BSZ:(c + 1) * BSZ],
                                     wqT[:, c * 512:(c + 1) * 512],
                                     start=(c == 0), stop=(c == NDC - 1))
                xq_sb = p1w.tile([BSZ, HL * HD], F32)
                nc.vector.tensor_copy(out=xq_sb, in_=xq_ps)
                if debug:
                    nc.sync.dma_start(out=dbg_xq[:, :], in_=xq_sb)

                # xqT: [128 d x 16 b] per local head
                xqT = p1w.tile([128, HL * BSZ], F32)
                for m in range(HL):
                    tp = p1ps.tile([128, BSZ], F32, tag="tp1")
                    nc.tensor.transpose(tp, xq_sb[:, m * 128:(m + 1) * 128],
                                        ident[0:BSZ, 0:BSZ])
                    nc.vector.tensor_copy(out=xqT[:, m * BSZ:(m + 1) * BSZ], in_=tp)

                # qt[h] = xq[:,h,:] @ wk_kv  (scaled) -> staged [64, 4096]
                # row layout = h_loc*16 + b
                for m in range(HL):
                    qstage = p1.tile([BSZ, DIM], F32, tag="qstage")
                    for j in range(8):
                        q_ps = p1ps.tile([BSZ, 512], F32, tag="qps")
                        nc.tensor.matmul(q_ps, xqT[:, m * BSZ:(m + 1) * BSZ],
                                         wk_sb[:, j * 512:(j + 1) * 512],
                                         start=True, stop=True)
                        nc.scalar.mul(
                            out=qstage[:, j * 512:(j + 1) * 512],
                            in_=q_ps, mul=SCALE)
                    d = nc.sync.dma_start(
                        out=a2a1_in.rearrange("(r b h) d -> h r b d",
                                              r=NC, b=B_LOC)[m],
                        in_=qstage)
                    stage_dmas1.append(d)
                    if debug and m == 0:
                        nc.sync.dma_start(out=dbg_qs[:, :], in_=qstage)


            if not nocc:
                cc1 = nc.gpsimd.collective_compute(
                    "AllToAll", mybir.AluOpType.bypass,
                    ins=[a2a1_in.opt()], outs=[a2a1_out.opt()], replica_groups=rg)
                for d in stage_dmas1:
                    add_dep_helper(cc1.ins, d.ins, reason="a2a1 input ready")

            # qT per local batch: [128 D x 32 h] per D-chunk
            # a2a1_out row = src_r*8 + b_loc*4 + h_loc ; head = 4*src_r + h_loc
            qT = [pers.tile([128, NDC * NH], F32, tag=f"qT{b}", name=f"qT{b}")
                  for b in range(B_LOC)]
            with (
                tc.tile_pool(name="qnat", bufs=2) as qnatp,
                tc.tile_pool(name="qnps", bufs=2, space="PSUM") as qnps,
            ):
                for b in range(B_LOC):
                    qnat = qnatp.tile([NH, DIM], F32, tag="qnat")
                    d = nc.sync.dma_start(
                        out=qnat,
                        in_=a2a1_out.rearrange("(r b h) d -> b r h d",
                                               r=NC, b=B_LOC)[b])
                    if not nocc:
                        add_dep_helper(d.ins, cc1.ins, reason="a2a1 done")
                    if debug:
                        nc.sync.dma_start(out=dbg_q[b * NH:(b + 1) * NH, :],
                                          in_=qnat)
                    for c in range(NDC):
                        tp = qnps.tile([128, NH], F32, tag="tpq")
                        nc.tensor.transpose(tp, qnat[:, c * 128:(c + 1) * 128],
                                            ident[0:NH, 0:NH])
                        nc.vector.tensor_copy(
                            out=qT[b][:, c * NH:(c + 1) * NH], in_=tp)

            # ---------------- Phase 2: streaming attention per local batch
            with (
                tc.tile_pool(name="xpool", bufs=6) as xpool,
                tc.tile_pool(name="xtpool", bufs=1) as xtpool,
                tc.tile_pool(name="attn", bufs=3) as apool,
                tc.tile_pool(name="small", bufs=2) as smallp,
                tc.tile_pool(name="ctx_sb", bufs=1) as ctxsbp,
                tc.tile_pool(name="tps", bufs=3, space="PSUM") as tps,
                tc.tile_pool(name="scps", bufs=2, space="PSUM") as scps,
                tc.tile_pool(name="ctxps", bufs=1, space="PSUM") as ctxps,
            ):
                for b in range(B_LOC):
                    ctx_ps = ctxps.tile([128, 1024], F32, tag="ctx")
                    sums = smallp.tile([NH, 4], F32, tag="sums")
                    for ch in range(4):
                        xts = []
                        for tt in range(4):
                            t = ch * 4 + tt
                            x_sb = xpool.tile([128, DIM], F32, tag="x",
                                              name=f"x{b}_{t}")
                            nc.sync.dma_start(
                                out=x_sb, in_=xp[b, t * 128:(t + 1) * 128, :])
                            xts.append(x_sb)
                        xtt = xtpool.tile([128, NDC * 512], F32, tag="xt")
                        xv = xtt.rearrange("p (c t) -> p c t", c=NDC)
                        for tt in range(4):
                            for cg in range(NDC // 4):
                                tp = tps.tile([128, 512], F32, tag="tp2")
                                for cc in range(4):
                                    c = cg * 4 + cc
                                    nc.tensor.transpose(
                                        tp[:, cc * 128:(cc + 1) * 128],
                                        xts[tt][:, c * 128:(c + 1) * 128], ident)
                                nc.vector.tensor_copy(
                                    out=xv[:, cg * 4:(cg + 1) * 4,
                                           tt * 128:(tt + 1) * 128],
                                    in_=tp.rearrange("p (a q) -> p a q", a=4))
                        if skel:
                            continue
                        sc_ps = scps.tile([NH, 512], F32, tag="sc")
                        for c in range(NDC):
                            nc.tensor.matmul(sc_ps,
                                             qT[b][:, c * NH:(c + 1) * NH],
                                             xtt[:, c * 512:(c + 1) * 512],
                                             start=(c == 0), stop=(c == NDC - 1))
                        attn_sb = apool.tile([NH, 512], F32, tag="attn")
                        nc.scalar.activation(out=attn_sb, in_=sc_ps,
                                             func=mybir.ActivationFunctionType.Exp,
                                             accum_out=sums[:, ch:ch + 1])
                        at_ps = tps.tile([128, 512], F32, tag="tp2")
                        for tt in range(4):
                            nc.tensor.transpose(
                                at_ps[:, tt * NH:(tt + 1) * NH],
                                attn_sb[:, tt * 128:(tt + 1) * 128],
                                ident[0:NH, 0:NH])
                        at_sb = apool.tile([128, 4 * NH], F32, tag="attnT")
                        nc.vector.tensor_copy(out=at_sb, in_=at_ps[:, 0:4 * NH])
                        for tt in range(4):
                            for g in range(4):
                                for jj in (g, g + 4):
                                    nc.tensor.matmul(
                                        ctx_ps[g * 32:(g + 1) * 32,
                                               (jj // 4) * 512:(jj // 4 + 1) * 512],
                                        at_sb[:, tt * NH:(tt + 1) * NH],
                                        xts[tt][:, jj * 512:(jj + 1) * 512],
                                        start=(ch == 0 and tt == 0),
                                        stop=(ch == 3 and tt == 3),
                                        tile_position=(0, g * 32))
                    # finalize batch: 1/rowsum, scale, stage for exchange
                    ssum = smallp.tile([NH, 1], F32, tag="ssum")
                    nc.vector.reduce_sum(out=ssum, in_=sums,
                                         axis=mybir.AxisListType.X)
                    rsum = smallp.tile([128, 1], F32, tag="rsum")
                    nc.vector.reciprocal(out=rsum[0:NH], in_=ssum)
                    for g in range(1, 4):
                        nc.vector.tensor_copy(out=rsum[g * 32:(g + 1) * 32],
                                              in_=rsum[0:NH])
                    ctx_sb = ctxsbp.tile([NH, DIM], F32, tag="ctxsb")
                    for j in range(8):
                        g = j % 4
                        nc.vector.tensor_scalar_mul(
                            ctx_sb[:, j * 512:(j + 1) * 512],
                            ctx_ps[g * 32:(g + 1) * 32,
                                   (j // 4) * 512:(j // 4 + 1) * 512],
                            rsum[g * 32:(g + 1) * 32, 0:1])
                    # ship: dest rank r gets heads 4r..4r+3 ; row r*8 + b*4 + h
                    d = nc.sync.dma_start(
                        out=a2a2_in.rearrange("(r b h) d -> r b h d",
                                              r=NC, b=B_LOC)[:, b],
                        in_=ctx_sb)
                    stage_dmas2.append(d)

            if not nocc:
                cc2 = nc.gpsimd.collective_compute(
                    "AllToAll", mybir.AluOpType.bypass,
                    ins=[a2a2_in.opt()], outs=[a2a2_out.opt()], replica_groups=rg)
                for d in stage_dmas2:
                    add_dep_helper(cc2.ins, d.ins, reason="a2a2 input ready")

            # ---------------- Phase 3: output projection (local heads, all b)
            with (
                tc.tile_pool(name="p3", bufs=2) as p3,
                tc.tile_pool(name="p3w", bufs=1) as p3w,
                tc.tile_pool(name="p3ps", bufs=2, space="PSUM") as p3ps,
            ):
                # a2a2_out row = src_r*8 + b_loc*4 + h = b*4 + h  (b=2*src_r+b_loc)
                ctxg = p3w.tile([BSZ * HL, DIM], F32)
                d = nc.sync.dma_start(out=ctxg, in_=a2a2_out[:, :])
                if not nocc:
                    add_dep_helper(d.ins, cc2.ins, reason="a2a2 done")
                if debug:
                    nc.sync.dma_start(out=dbg_ctx[:, :], in_=ctxg)
                ctxgT = p3w.tile([128, NDC * BSZ * HL], F32)
                for c in range(NDC):
                    tp = p3ps.tile([128, 128], F32, tag="tp3")
                    nc.tensor.transpose(tp[:, 0:BSZ * HL],
                                        ctxg[:, c * 128:(c + 1) * 128],
                                        ident[0:BSZ * HL, 0:BSZ * HL])
                    nc.vector.tensor_copy(out=ctxgT[:, c * 64:(c + 1) * 64],
                                          in_=tp[:, 0:64])

                wv_sb = p3w.tile([HD, DIM], F32)
                nc.sync.dma_start(out=wv_sb, in_=wv[:, :])
                wvT = p3w.tile([128, DIM], F32)
                for c in range(NDC):
                    tp = p3ps.tile([128, 128], F32, tag="tp3")
                    nc.tensor.transpose(tp, wv_sb[:, c * 128:(c + 1) * 128], ident)
                    nc.vector.tensor_copy(out=wvT[:, c * 128:(c + 1) * 128], in_=tp)

                # out[b*4+h, d] accumulation over D-chunks
                op_ps = p3ps.tile([BSZ * HL, HD], F32, tag="op")
                for c in range(NDC):
                    nc.tensor.matmul(op_ps, ctxgT[:, c * 64:(c + 1) * 64],
                                     wvT[:, c * 128:(c + 1) * 128],
                                     start=(c == 0), stop=(c == NDC - 1))
                op_sb = p3w.tile([BSZ * HL, HD], F32)
                nc.vector.tensor_copy(out=op_sb, in_=op_ps)
                # outT [128 d x 64 (b*4+h)]
                otp = p3ps.tile([128, 128], F32, tag="tp3")
                nc.tensor.transpose(otp[:, 0:64], op_sb,
                                    ident[0:BSZ * HL, 0:BSZ * HL])
                outT = p3w.tile([128, BSZ * HL], F32)
                nc.vector.tensor_copy(out=outT, in_=otp[:, 0:64])

                # woT: per h_rel m: [128 d x 4096 j]
                woT = p3w.tile([128, HL * DIM], F32)
                for jt in range(NDC):
                    wo_sb = p3.tile([128, HL * HD], F32, tag="wonat")
                    nc.sync.dma_start(out=wo_sb, in_=wo[jt * 128:(jt + 1) * 128, :])
                    for m in range(HL):
                        tp = p3ps.tile([128, 128], F32, tag="tp3")
                        nc.tensor.transpose(tp, wo_sb[:, m * 128:(m + 1) * 128],
                                            ident)
                        nc.vector.tensor_copy(
                            out=woT[:, m * DIM + jt * 128: m * DIM + (jt + 1) * 128],
                            in_=tp)

                # y partial [16 b, 4096 j]
                y_sb = p3w.tile([BSZ, DIM], F32)
                oT = outT.rearrange("p (b h) -> p h b", h=HL)
                for jc in range(8):
                    y_ps = p3ps.tile([BSZ, 512], F32, tag="yps")
                    for m in range(HL):
                        nc.tensor.matmul(y_ps, oT[:, m, :],
                                         woT[:, m * DIM + jc * 512:
                                             m * DIM + (jc + 1) * 512],
                                         start=(m == 0), stop=(m == HL - 1))
                    nc.vector.tensor_copy(out=y_sb[:, jc * 512:(jc + 1) * 512],
                                          in_=y_ps)
                nc.sync.dma_start(out=y[:, :], in_=y_sb)

    nc.finalize()
    return nc


_PROGRAM_CACHE = {}


def kernel(x_pre, wq, wk, wv, wo, _trace=False, _tmpdir=None):
    x_pre = np.ascontiguousarray(np.asarray(x_pre, dtype=np.float32))
    wq = np.asarray(wq, dtype=np.float32)
    wk = np.asarray(wk, dtype=np.float32)
    wv = np.asarray(wv, dtype=np.float32)
    wo = np.asarray(wo, dtype=np.float32)

    if "nc" not in _PROGRAM_CACHE:
        _PROGRAM_CACHE["nc"] = build_program()
    nc = _PROGRAM_CACHE["nc"]

    xl = np.ascontiguousarray(x_pre[:, -1, :])
    in_maps = []
    for i in range(NC):
        in_maps.append({
            "xp": np.ascontiguousarray(x_pre[2 * i:2 * i + 2]),
            "xl": xl,
            "wq": np.ascontiguousarray(wq[512 * i:512 * (i + 1), :]),
            "wk": np.ascontiguousarray(wk[128 * i:128 * (i + 1), :]),
            "wv": np.ascontiguousarray(wv[128 * i:128 * (i + 1), :]),
            "wo": np.ascontiguousarray(wo[:, 512 * i:512 * (i + 1)]),
        })

    kwargs = {}
    if _trace:
        kwargs = dict(trace=True, trace_cores=[0])
    if _tmpdir is not None:
        kwargs["tmpdir"] = _tmpdir
    res = run_bass_kernel_spmd(nc, in_maps, core_ids=list(range(NC)), **kwargs)
    y = np.zeros((BSZ, DIM), np.float32)
    for i in range(NC):
        y += res.results[i]["y"]
    if _trace:
        print("HW exec time:", res.exec_time_ns, "ns")
    return y.reshape(BSZ, 1, DIM)
